# revision 12
# baseline (speedup 1.0000x reference)
"""Trainium2 Bass kernel for nn_Discriminator (segment_reduce, 8 cores).

Math (collapsed form of the reference):
  The reference projects the full embedding table (emb = E @ W_i.T + b_i),
  gathers pos/neg rows, does a segment-mean over pos rows, and scores each
  row with a bilinear form against its segment embedding.  Everything is
  linear, so it collapses to operations on RAW embedding rows:

    m[s]     = mean of raw E rows of segment s's pos samples        [256]
    grid[s]  = W_i m[s] + b_i
    h[s]     = Wb grid[s]                  (Wb = W_k[0])
    u[s]     = W_i^T h[s];   c[s] = b_i . h[s] + b_k
    logit[n] = E[idx[n]] . u[seg(n)] + c[seg(n)]

  So the device only gathers raw rows once (pos rows reused from SBUF for
  both the segment mean and the dot), plus tiny 256x256 matmuls on 1024
  segment vectors.  Memory traffic ~= one 1KB row per sample: ~805 MB
  total across 8 cores vs ~1.6 GB for the reference order.

Sharding: data-parallel over samples, segments kept whole per core
(core k owns segments [k*128, (k+1)*128), i.e. pos rows [k*16384, ...)
and neg rows [k*81920, ...)).  Fully local, no collectives.

Device pipeline per core:
  - 16 indirect gathers stream pos rows into 8 resident SBUF tiles;
    80 indirect gathers stream neg rows through a 5-deep tile pool.
  - Segment means are computed transposed (PE contracts the partition
    axis of each 128-row block against a 1/seg_size column).
  - The tiny u-chain runs per group of 16 segments so dot products can
    start as soon as the first pos tiles land.
  - Per segment: u-row staged to partition 0 (SBUF->SBUF DMA), PE
    broadcasts it to 128 partitions in PSUM, ACT copies it to SBUF, and
    DVE does one fused multiply+reduce (tensor_tensor_reduce) per
    128-row block -> logits column.
"""

import numpy as np

import concourse.bass as bass
import concourse.bacc as bacc
import concourse.mybir as mybir
from concourse import bass_utils
from concourse.masks import make_identity
from concourse.tile import TileContext

F32 = mybir.dt.float32
I32 = mybir.dt.int32

N_NODES = 200000
H = 256
N_SEG = 1024
SEG_SZ = 128          # rows per segment (asserted at runtime)
N_POS = N_SEG * SEG_SZ          # 131072
NEG_RATIO = 5
N_NEG = N_POS * NEG_RATIO       # 655360
N_CORES = 8

SEG_PC = N_SEG // N_CORES       # 128 segments per core
POS_PC = N_POS // N_CORES       # 16384
NEG_PC = N_NEG // N_CORES       # 81920
P = 128
POS_BLK = POS_PC // P           # 128 blocks (block == segment for pos)
NEG_BLK = NEG_PC // P           # 640 blocks (5 consecutive per segment)
TOT_BLK = POS_BLK + NEG_BLK     # 768 logit columns

GB = 16                         # blocks per indirect-gather call
NEG_BUFS = 3                    # in-flight neg gather tiles
GSEG = 16                       # segments per u-chain group
NGRP = SEG_PC // GSEG           # 8 groups

_CACHED = None


def _build_module() -> bass.Bass:
    # Bacc (not raw Bass): its compile() pass splits multi-sem waits into
    # event semaphores — walrus rejects >1 sync wait per instruction.
    nc = bacc.Bacc("TRN2", target_bir_lowering=False, debug=False)

    table = nc.dram_tensor("table", [N_NODES, H], F32, kind="ExternalInput")
    pos_idx = nc.dram_tensor("pos_idx", [P, POS_BLK], I32, kind="ExternalInput")
    neg_idx = nc.dram_tensor("neg_idx", [P, NEG_BLK], I32, kind="ExternalInput")
    # w_iT[p, j, f'] = W_i.T[j*128+p, f']     (lhsT tiles for G = W_i @ M)
    w_iT = nc.dram_tensor("w_iT", [P, 2, H], F32, kind="ExternalInput")
    # wbT[p, j, d]  = Wb.T[j*128+p, d]        (lhsT tiles for H = Wb @ G)
    wbT = nc.dram_tensor("wbT", [P, 2, H], F32, kind="ExternalInput")
    # w_ext[p, j, m] = [W_i | b_i][j*128+p, m]  (lhsT tiles for U~ = W_ext^T H)
    w_ext = nc.dram_tensor("w_ext", [P, 2, H + 1], F32, kind="ExternalInput")
    b_i2 = nc.dram_tensor("b_i2", [P, 2], F32, kind="ExternalInput")
    b_k = nc.dram_tensor("b_k", [1, 1], F32, kind="ExternalInput")
    inv_sz = nc.dram_tensor("inv_sz", [P, SEG_PC], F32, kind="ExternalInput")
    logits_d = nc.dram_tensor("logits", [P, TOT_BLK], F32, kind="ExternalOutput")

    W1 = H + 1

    with TileContext(nc) as tc:
        with (
            tc.tile_pool(name="const", bufs=1) as const,
            tc.tile_pool(name="grp", bufs=2) as grp,
            tc.tile_pool(name="pospool", bufs=NGRP) as pospool,
            tc.tile_pool(name="negpool", bufs=NEG_BUFS) as negpool,
            tc.tile_pool(name="scratch", bufs=2) as scratch,
            tc.tile_pool(name="ustage", bufs=3) as ustagep,
            tc.tile_pool(name="ubcsb", bufs=3) as ubcsbp,
            tc.tile_pool(name="mt", bufs=2, space="PSUM") as mtp,
            tc.tile_pool(name="chain", bufs=4, space="PSUM") as chainp,
            tc.tile_pool(name="ubc", bufs=2, space="PSUM") as ubcp,
        ):
            # ---- constants / weights ----
            ident = const.tile([P, P], F32, tag="ident")
            make_identity(nc, ident[:])
            ones1 = const.tile([1, P], F32, tag="ones1")
            nc.gpsimd.memset(ones1[:], 1.0)
            one11 = const.tile([1, 1], F32, tag="one11")
            nc.gpsimd.memset(one11[:], 1.0)

            pos_idx_sb = const.tile([P, POS_BLK], I32, tag="pidx")
            nc.sync.dma_start(pos_idx_sb[:], pos_idx[:, :])
            neg_idx_sb = const.tile([P, NEG_BLK], I32, tag="nidx")
            nc.sync.dma_start(neg_idx_sb[:], neg_idx[:, :])
            w_iT_sb = const.tile([P, 2 * H], F32, tag="wiT")
            nc.sync.dma_start(w_iT_sb[:], w_iT[:, :, :])
            wbT_sb = const.tile([P, 2 * H], F32, tag="wbT")
            nc.sync.dma_start(wbT_sb[:], wbT[:, :, :])
            w_ext_sb = const.tile([P, 2 * W1], F32, tag="wext")
            nc.sync.dma_start(w_ext_sb[:], w_ext[:, :, :])
            b_i2_sb = const.tile([P, 2], F32, tag="bi2")
            nc.sync.dma_start(b_i2_sb[:], b_i2[:, :])
            b_k_sb = const.tile([1, 1], F32, tag="bk")
            nc.sync.dma_start(b_k_sb[:], b_k[:, :])
            inv_sb = const.tile([P, SEG_PC], F32, tag="inv")
            nc.sync.dma_start(inv_sb[:], inv_sz[:, :])

            logits_sb = const.tile([P, TOT_BLK], F32, tag="logits")

            # ---- gathers, interleaved so neg data flows while the u-chain
            # of early groups is still being computed.  Pos gathers for a
            # group are emitted ~2 groups ahead of the neg gathers whose
            # dots will need that group's u vectors.
            pos_tiles = [None] * NGRP
            neg_tiles = [None] * (NEG_BLK // GB)

            def emit_pos_group(g):
                pt = pospool.tile([P, GSEG * H], F32, tag="pos")
                pos_tiles[g] = pt
                for half in range(GSEG // GB):
                    nc.gpsimd.indirect_dma_start(
                        out=pt[:, half * GB * H:(half + 1) * GB * H],
                        out_offset=None,
                        in_=table[:, :],
                        in_offset=bass.IndirectOffsetOnAxis(
                            ap=pos_idx_sb[:, g * GSEG + half * GB:
                                          g * GSEG + (half + 1) * GB],
                            axis=0,
                        ),
                    )

            def emit_neg(gi):
                t = negpool.tile([P, GB * H], F32, tag="neg")
                neg_tiles[gi] = t
                nc.gpsimd.indirect_dma_start(
                    out=t[:, :],
                    out_offset=None,
                    in_=table[:, :],
                    in_offset=bass.IndirectOffsetOnAxis(
                        ap=neg_idx_sb[:, gi * GB:(gi + 1) * GB], axis=0
                    ),
                )

            NEG_PER_GRP = NEG_BLK // GB // NGRP         # 5 neg calls per group
            emit_pos_group(0)
            emit_pos_group(1)
            for g in range(NGRP):
                for i in range(NEG_PER_GRP):
                    emit_neg(g * NEG_PER_GRP + i)
                    if i == 2 and g + 2 < NGRP:
                        emit_pos_group(g + 2)

            # ---- per group of GSEG segments: means + u-chain + dots ----
            for g in range(NGRP):
                pt = pos_tiles[g]
                s0 = g * GSEG

                # segment means, directly transposed: psum_mt[t][f, s_loc]
                psum_mt = []
                for _t in range(2):
                    pmt = mtp.tile([P, GSEG], F32, tag="mt")
                    psum_mt.append(pmt)
                for bl in range(GSEG):
                    for t in range(2):
                        nc.tensor.matmul(
                            out=psum_mt[t][:, bl:bl + 1],
                            lhsT=pt[:, bl * H + t * P: bl * H + t * P + P],
                            rhs=inv_sb[:, s0 + bl:s0 + bl + 1],
                            start=True,
                            stop=True,
                        )
                mT = grp.tile([P, 2 * GSEG], F32, tag="mT")
                for t in range(2):
                    nc.vector.tensor_copy(
                        mT[:, t * GSEG:(t + 1) * GSEG], psum_mt[t][:])

                # G_T = W_i @ M_T + b_i
                gT = grp.tile([P, 2 * GSEG], F32, tag="gT")
                for t in range(2):
                    pg = chainp.tile([P, GSEG], F32, tag="chain")
                    for j in range(2):
                        nc.tensor.matmul(
                            out=pg[:],
                            lhsT=w_iT_sb[:, j * H + t * P: j * H + t * P + P],
                            rhs=mT[:, j * GSEG:(j + 1) * GSEG],
                            start=(j == 0),
                            stop=(j == 1),
                        )
                    nc.vector.tensor_scalar(
                        out=gT[:, t * GSEG:(t + 1) * GSEG], in0=pg[:],
                        scalar1=b_i2_sb[:, t:t + 1], scalar2=None,
                        op0=mybir.AluOpType.add,
                    )

                # H_T = Wb @ G_T
                hT = grp.tile([P, 2 * GSEG], F32, tag="hT")
                for t in range(2):
                    ph = chainp.tile([P, GSEG], F32, tag="chain")
                    for j in range(2):
                        nc.tensor.matmul(
                            out=ph[:],
                            lhsT=wbT_sb[:, j * H + t * P: j * H + t * P + P],
                            rhs=gT[:, j * GSEG:(j + 1) * GSEG],
                            start=(j == 0),
                            stop=(j == 1),
                        )
                    nc.vector.tensor_copy(hT[:, t * GSEG:(t + 1) * GSEG], ph[:])

                # U~_T = [W_i | b_i]^T @ H_T, then transpose to rows
                u_rows = grp.tile([GSEG, W1], F32, tag="urows")
                for t in range(2):
                    pu = chainp.tile([P, GSEG], F32, tag="chain")
                    for j in range(2):
                        nc.tensor.matmul(
                            out=pu[:],
                            lhsT=w_ext_sb[:, j * W1 + t * P: j * W1 + t * P + P],
                            rhs=hT[:, j * GSEG:(j + 1) * GSEG],
                            start=(j == 0),
                            stop=(j == 1),
                        )
                    usb = grp.tile([P, GSEG], F32, tag=f"u{t}")
                    nc.vector.tensor_copy(usb[:], pu[:])
                    ptr = chainp.tile([GSEG, P], F32, tag="chain")
                    nc.tensor.transpose(ptr[:], usb[:], ident[:])
                    nc.vector.tensor_copy(u_rows[:, t * P:(t + 1) * P], ptr[:])
                # c row: [1, GSEG] -> +b_k -> transpose -> column 256
                puc = chainp.tile([1, GSEG], F32, tag="chain")
                for j in range(2):
                    nc.tensor.matmul(
                        out=puc[:],
                        lhsT=w_ext_sb[:, j * W1 + H: j * W1 + H + 1],
                        rhs=hT[:, j * GSEG:(j + 1) * GSEG],
                        start=(j == 0),
                        stop=(j == 1),
                    )
                uc_sb = grp.tile([1, GSEG], F32, tag="ucsb")
                nc.vector.tensor_scalar(
                    out=uc_sb[:], in0=puc[:], scalar1=b_k_sb[:1, :1],
                    scalar2=None, op0=mybir.AluOpType.add,
                )
                ptc = chainp.tile([GSEG, 1], F32, tag="chain")
                nc.tensor.transpose(ptc[:], uc_sb[:], one11[:])
                nc.vector.tensor_copy(u_rows[:, H:H + 1], ptc[:])

                # per segment: stage u-row to partition 0, PE-broadcast to
                # PSUM, ACT-copy to SBUF, then one fused dot per block.
                for sl in range(GSEG):
                    s = s0 + sl
                    ustage = ustagep.tile([1, W1], F32, tag="ustage")
                    nc.scalar.dma_start(ustage[:], u_rows[sl:sl + 1, :])
                    pub = ubcp.tile([P, W1], F32, tag="ubc")
                    nc.tensor.matmul(
                        out=pub[:], lhsT=ones1[:], rhs=ustage[:1, :],
                        start=True, stop=True,
                    )
                    ubs = ubcsbp.tile([P, W1], F32, tag="ubs")
                    nc.scalar.copy(out=ubs[:], in_=pub[:])
                    blocks = [("pos", sl, s)]
                    for q in range(NEG_RATIO * s, NEG_RATIO * (s + 1)):
                        blocks.append(("neg", q, POS_BLK + q))
                    for kind, b, col in blocks:
                        if kind == "pos":
                            in0 = pt[:, b * H:(b + 1) * H]
                        else:
                            in0 = neg_tiles[b // GB][
                                :, (b % GB) * H:(b % GB + 1) * H]
                        sc = scratch.tile([P, H], F32, tag="ttr")
                        nc.vector.tensor_tensor_reduce(
                            out=sc[:],
                            in0=in0,
                            in1=ubs[:, 0:H],
                            scale=1.0,
                            scalar=ubs[:, H:H + 1],
                            op0=mybir.AluOpType.mult,
                            op1=mybir.AluOpType.add,
                            accum_out=logits_sb[:, col:col + 1],
                        )

            nc.sync.dma_start(logits_d[:, :], logits_sb[:])

    nc.compile()
    return nc


def get_module() -> bass.Bass:
    global _CACHED
    if _CACHED is None:
        _CACHED = _build_module()
    return _CACHED


def make_in_maps(inputs: dict) -> list[dict]:
    emb = np.ascontiguousarray(np.asarray(inputs["embedding"], dtype=np.float32))
    gs = np.asarray(inputs["grid_sizes"]).astype(np.int64)
    pos_s = np.asarray(inputs["pos_samples"]).astype(np.int32)
    neg_s = np.asarray(inputs["neg_samples"]).astype(np.int32)
    W_i = np.asarray(inputs["W_i"], dtype=np.float32)
    b_i = np.asarray(inputs["b_i"], dtype=np.float32)
    Wb = np.asarray(inputs["W_k"], dtype=np.float32)[0]
    b_kv = np.asarray(inputs["b_k"], dtype=np.float32)

    if not (gs.shape == (N_SEG,) and np.all(gs == SEG_SZ)):
        raise RuntimeError("kernel assumes grid_sizes == 128 everywhere")
    assert pos_s.shape == (N_POS,) and neg_s.shape == (N_NEG,)

    w_iT_np = np.ascontiguousarray(
        W_i.T.reshape(2, P, H).transpose(1, 0, 2))
    wbT_np = np.ascontiguousarray(
        Wb.T.reshape(2, P, H).transpose(1, 0, 2))
    W_ext = np.concatenate([W_i, b_i[:, None]], axis=1)        # [256, 257]
    w_ext_np = np.ascontiguousarray(
        W_ext.reshape(2, P, H + 1).transpose(1, 0, 2))
    b_i2_np = np.ascontiguousarray(b_i.reshape(2, P).T)
    b_k_np = b_kv.reshape(1, 1)

    in_maps = []
    for k in range(N_CORES):
        pk = pos_s[k * POS_PC:(k + 1) * POS_PC].reshape(POS_BLK, P).T
        nk = neg_s[k * NEG_PC:(k + 1) * NEG_PC].reshape(NEG_BLK, P).T
        inv_k = np.broadcast_to(
            (1.0 / gs[k * SEG_PC:(k + 1) * SEG_PC].astype(np.float64)
             ).astype(np.float32)[None, :], (P, SEG_PC))
        in_maps.append({
            "table": emb,
            "pos_idx": np.ascontiguousarray(pk),
            "neg_idx": np.ascontiguousarray(nk),
            "w_iT": w_iT_np,
            "wbT": wbT_np,
            "w_ext": w_ext_np,
            "b_i2": b_i2_np,
            "b_k": b_k_np,
            "inv_sz": np.ascontiguousarray(inv_k),
        })
    return in_maps


def assemble_output(core_outs: list[np.ndarray]) -> np.ndarray:
    pos_parts, neg_parts = [], []
    for k in range(N_CORES):
        o = np.asarray(core_outs[k])
        assert o.shape == (P, TOT_BLK)
        pos_parts.append(o[:, :POS_BLK].T.ravel())
        neg_parts.append(o[:, POS_BLK:].T.ravel())
    return np.concatenate(pos_parts + neg_parts).astype(np.float32)


def kernel(**inputs) -> np.ndarray:
    nc = get_module()
    in_maps = make_in_maps(inputs)
    res = bass_utils.run_bass_kernel_spmd(
        nc, in_maps, core_ids=list(range(N_CORES)))
    return assemble_output([r["logits"] for r in res.results])


# revision 18
# speedup vs baseline: 1.0987x; 1.0987x over previous
"""Trainium2 Bass kernel for nn_Discriminator (segment_reduce, 8 cores).

Math (collapsed form of the reference):
  The reference projects the full embedding table (emb = E @ W_i.T + b_i),
  gathers pos/neg rows, does a segment-mean over pos rows, and scores each
  row with a bilinear form against its segment embedding.  Everything is
  linear, so it collapses to operations on RAW embedding rows:

    m[s]     = mean of raw E rows of segment s's pos samples        [256]
    grid[s]  = W_i m[s] + b_i
    h[s]     = Wb grid[s]                  (Wb = W_k[0])
    u[s]     = W_i^T h[s];   c[s] = b_i . h[s] + b_k
    logit[n] = E[idx[n]] . u[seg(n)] + c[seg(n)]

  So the device only gathers raw rows once (pos rows reused from SBUF for
  both the segment mean and the dot), plus tiny 256x256 matmuls on 1024
  segment vectors.  Memory traffic ~= one 1KB row per sample: ~805 MB
  total across 8 cores vs ~1.6 GB for the reference order.

Sharding: data-parallel over samples, segments kept whole per core
(core k owns segments [k*128, (k+1)*128), i.e. pos rows [k*16384, ...)
and neg rows [k*81920, ...)).  Fully local, no collectives.

Device pipeline per core:
  - 16 indirect gathers stream pos rows into 8 resident SBUF tiles;
    80 indirect gathers stream neg rows through a 5-deep tile pool.
  - Segment means are computed transposed (PE contracts the partition
    axis of each 128-row block against a 1/seg_size column).
  - The tiny u-chain runs per group of 16 segments so dot products can
    start as soon as the first pos tiles land.
  - Per segment: u-row staged to partition 0 (SBUF->SBUF DMA), PE
    broadcasts it to 128 partitions in PSUM, ACT copies it to SBUF, and
    DVE does one fused multiply+reduce (tensor_tensor_reduce) per
    128-row block -> logits column.
"""

import numpy as np

import concourse.bass as bass
import concourse.bacc as bacc
import concourse.mybir as mybir
from concourse import bass_utils
from concourse.masks import make_identity
from concourse.tile import TileContext

F32 = mybir.dt.float32
I32 = mybir.dt.int32

N_NODES = 200000
H = 256
N_SEG = 1024
SEG_SZ = 128          # rows per segment (asserted at runtime)
N_POS = N_SEG * SEG_SZ          # 131072
NEG_RATIO = 5
N_NEG = N_POS * NEG_RATIO       # 655360
N_CORES = 8

SEG_PC = N_SEG // N_CORES       # 128 segments per core
POS_PC = N_POS // N_CORES       # 16384
NEG_PC = N_NEG // N_CORES       # 81920
P = 128
POS_BLK = POS_PC // P           # 128 blocks (block == segment for pos)
NEG_BLK = NEG_PC // P           # 640 blocks (5 consecutive per segment)
TOT_BLK = POS_BLK + NEG_BLK     # 768 logit columns

GB = 16                         # blocks per dma_gather call (2048 rows)
NEG_BUFS = 3                    # in-flight neg gather tiles
GSEG = 16                       # segments per u-chain group
NGRP = SEG_PC // GSEG           # 8 groups

# dma_gather uses int16 local indices, so the host packs each core's rows
# into 3 windows of <=32768 unique rows (one window per 32768 sample
# positions; uniques can never exceed positions, so they always fit).
WIN_POS = 32768                 # sample positions per window
N_WIN = (POS_PC + NEG_PC) // WIN_POS        # 3
CALL_IDX = 2048                 # indices per gather call (GB blocks)
CALLS_PER_WIN = WIN_POS // CALL_IDX         # 16
N_CALLS = N_WIN * CALLS_PER_WIN             # 48
IDX_COLS = CALL_IDX // 16                   # wrapped int16 columns per call

_CACHED = None


def _build_module() -> bass.Bass:
    # Bacc (not raw Bass): its compile() pass splits multi-sem waits into
    # event semaphores — walrus rejects >1 sync wait per instruction.
    nc = bacc.Bacc("TRN2", target_bir_lowering=False, debug=False)

    table = nc.dram_tensor("table", [N_WIN * WIN_POS, H], F32,
                           kind="ExternalInput")
    idx16 = nc.dram_tensor("idx16", [P, N_CALLS * IDX_COLS], mybir.dt.int16,
                           kind="ExternalInput")
    # w_iT[p, j, f'] = W_i.T[j*128+p, f']     (lhsT tiles for G = W_i @ M)
    w_iT = nc.dram_tensor("w_iT", [P, 2, H], F32, kind="ExternalInput")
    # wbT[p, j, d]  = Wb.T[j*128+p, d]        (lhsT tiles for H = Wb @ G)
    wbT = nc.dram_tensor("wbT", [P, 2, H], F32, kind="ExternalInput")
    # w_ext[p, j, m] = [W_i | b_i][j*128+p, m]  (lhsT tiles for U~ = W_ext^T H)
    w_ext = nc.dram_tensor("w_ext", [P, 2, H + 1], F32, kind="ExternalInput")
    b_i2 = nc.dram_tensor("b_i2", [P, 2], F32, kind="ExternalInput")
    b_k = nc.dram_tensor("b_k", [1, 1], F32, kind="ExternalInput")
    inv_sz = nc.dram_tensor("inv_sz", [P, SEG_PC], F32, kind="ExternalInput")
    logits_d = nc.dram_tensor("logits", [P, TOT_BLK], F32, kind="ExternalOutput")

    W1 = H + 1

    with TileContext(nc) as tc:
        with (
            tc.tile_pool(name="const", bufs=1) as const,
            tc.tile_pool(name="grp", bufs=2) as grp,
            tc.tile_pool(name="pospool", bufs=NGRP) as pospool,
            tc.tile_pool(name="negpool", bufs=NEG_BUFS) as negpool,
            tc.tile_pool(name="scratch", bufs=1) as scratch,
            tc.tile_pool(name="ustage", bufs=2) as ustagep,
            tc.tile_pool(name="ubcsb", bufs=2) as ubcsbp,
            tc.tile_pool(name="mt", bufs=2, space="PSUM") as mtp,
            tc.tile_pool(name="chain", bufs=4, space="PSUM") as chainp,
            tc.tile_pool(name="ubc", bufs=2, space="PSUM") as ubcp,
        ):
            # ---- constants / weights ----
            ident = const.tile([P, P], F32, tag="ident")
            make_identity(nc, ident[:])
            ones1 = const.tile([1, P], F32, tag="ones1")
            nc.gpsimd.memset(ones1[:], 1.0)
            one11 = const.tile([1, 1], F32, tag="one11")
            nc.gpsimd.memset(one11[:], 1.0)

            idx16_sb = const.tile([P, N_CALLS * IDX_COLS], mybir.dt.int16,
                                  tag="idx16")
            nc.sync.dma_start(idx16_sb[:], idx16[:, :])
            w_iT_sb = const.tile([P, 2 * H], F32, tag="wiT")
            nc.sync.dma_start(w_iT_sb[:], w_iT[:, :, :])
            wbT_sb = const.tile([P, 2 * H], F32, tag="wbT")
            nc.sync.dma_start(wbT_sb[:], wbT[:, :, :])
            w_ext_sb = const.tile([P, 2 * W1], F32, tag="wext")
            nc.sync.dma_start(w_ext_sb[:], w_ext[:, :, :])
            b_i2_sb = const.tile([P, 2], F32, tag="bi2")
            nc.sync.dma_start(b_i2_sb[:], b_i2[:, :])
            b_k_sb = const.tile([1, 1], F32, tag="bk")
            nc.sync.dma_start(b_k_sb[:], b_k[:, :])
            inv_sb = const.tile([P, SEG_PC], F32, tag="inv")
            nc.sync.dma_start(inv_sb[:], inv_sz[:, :])

            logits_sb = const.tile([P, TOT_BLK], F32, tag="logits")

            # ---- gathers (dma_gather, 2048 rows per call), interleaved so
            # neg data flows while the u-chain of early groups is computed.
            # Call ci covers global blocks [16ci, 16ci+16): calls 0..7 are
            # the pos groups, 8..47 the neg tiles.  Gathered row for list
            # position i lands at (partition i%128, block i//128).
            pos_tiles = [None] * NGRP
            neg_tiles = [None] * (NEG_BLK // GB)

            def emit_gather(ci, tile_):
                w = ci // CALLS_PER_WIN
                nc.gpsimd.dma_gather(
                    out_ap=tile_[:, :].rearrange("p (b h) -> p b h", b=GB),
                    in_ap=table[w * WIN_POS:(w + 1) * WIN_POS, :],
                    idxs_ap=idx16_sb[:, ci * IDX_COLS:(ci + 1) * IDX_COLS],
                    num_idxs=CALL_IDX,
                    num_idxs_reg=CALL_IDX,
                    elem_size=H,
                )

            def emit_pos_group(g):
                pt = pospool.tile([P, GSEG * H], F32, tag="pos")
                pos_tiles[g] = pt
                emit_gather(g, pt)

            def emit_neg(gi):
                t = negpool.tile([P, GB * H], F32, tag="neg")
                neg_tiles[gi] = t
                emit_gather(NGRP + gi, t)

            NEG_PER_GRP = NEG_BLK // GB // NGRP         # 5 neg calls per group
            emit_pos_group(0)
            emit_pos_group(1)
            for g in range(NGRP):
                for i in range(NEG_PER_GRP):
                    emit_neg(g * NEG_PER_GRP + i)
                    if i == 2 and g + 2 < NGRP:
                        emit_pos_group(g + 2)

            # ---- per group of GSEG segments: means + u-chain + dots ----
            for g in range(NGRP):
                pt = pos_tiles[g]
                s0 = g * GSEG

                # segment means, directly transposed: psum_mt[t][f, s_loc]
                psum_mt = []
                for _t in range(2):
                    pmt = mtp.tile([P, GSEG], F32, tag="mt")
                    psum_mt.append(pmt)
                for bl in range(GSEG):
                    for t in range(2):
                        nc.tensor.matmul(
                            out=psum_mt[t][:, bl:bl + 1],
                            lhsT=pt[:, bl * H + t * P: bl * H + t * P + P],
                            rhs=inv_sb[:, s0 + bl:s0 + bl + 1],
                            start=True,
                            stop=True,
                        )
                mT = grp.tile([P, 2 * GSEG], F32, tag="mT")
                for t in range(2):
                    nc.vector.tensor_copy(
                        mT[:, t * GSEG:(t + 1) * GSEG], psum_mt[t][:])

                # G_T = W_i @ M_T + b_i
                gT = grp.tile([P, 2 * GSEG], F32, tag="gT")
                for t in range(2):
                    pg = chainp.tile([P, GSEG], F32, tag="chain")
                    for j in range(2):
                        nc.tensor.matmul(
                            out=pg[:],
                            lhsT=w_iT_sb[:, j * H + t * P: j * H + t * P + P],
                            rhs=mT[:, j * GSEG:(j + 1) * GSEG],
                            start=(j == 0),
                            stop=(j == 1),
                        )
                    nc.vector.tensor_scalar(
                        out=gT[:, t * GSEG:(t + 1) * GSEG], in0=pg[:],
                        scalar1=b_i2_sb[:, t:t + 1], scalar2=None,
                        op0=mybir.AluOpType.add,
                    )

                # H_T = Wb @ G_T
                hT = grp.tile([P, 2 * GSEG], F32, tag="hT")
                for t in range(2):
                    ph = chainp.tile([P, GSEG], F32, tag="chain")
                    for j in range(2):
                        nc.tensor.matmul(
                            out=ph[:],
                            lhsT=wbT_sb[:, j * H + t * P: j * H + t * P + P],
                            rhs=gT[:, j * GSEG:(j + 1) * GSEG],
                            start=(j == 0),
                            stop=(j == 1),
                        )
                    nc.vector.tensor_copy(hT[:, t * GSEG:(t + 1) * GSEG], ph[:])

                # U~_T = [W_i | b_i]^T @ H_T, then transpose to rows
                u_rows = grp.tile([GSEG, W1], F32, tag="urows")
                for t in range(2):
                    pu = chainp.tile([P, GSEG], F32, tag="chain")
                    for j in range(2):
                        nc.tensor.matmul(
                            out=pu[:],
                            lhsT=w_ext_sb[:, j * W1 + t * P: j * W1 + t * P + P],
                            rhs=hT[:, j * GSEG:(j + 1) * GSEG],
                            start=(j == 0),
                            stop=(j == 1),
                        )
                    usb = grp.tile([P, GSEG], F32, tag=f"u{t}")
                    nc.vector.tensor_copy(usb[:], pu[:])
                    ptr = chainp.tile([GSEG, P], F32, tag="chain")
                    nc.tensor.transpose(ptr[:], usb[:], ident[:])
                    nc.vector.tensor_copy(u_rows[:, t * P:(t + 1) * P], ptr[:])
                # c row: [1, GSEG] -> +b_k -> transpose -> column 256
                puc = chainp.tile([1, GSEG], F32, tag="chain")
                for j in range(2):
                    nc.tensor.matmul(
                        out=puc[:],
                        lhsT=w_ext_sb[:, j * W1 + H: j * W1 + H + 1],
                        rhs=hT[:, j * GSEG:(j + 1) * GSEG],
                        start=(j == 0),
                        stop=(j == 1),
                    )
                uc_sb = grp.tile([1, GSEG], F32, tag="ucsb")
                nc.vector.tensor_scalar(
                    out=uc_sb[:], in0=puc[:], scalar1=b_k_sb[:1, :1],
                    scalar2=None, op0=mybir.AluOpType.add,
                )
                ptc = chainp.tile([GSEG, 1], F32, tag="chain")
                nc.tensor.transpose(ptc[:], uc_sb[:], one11[:])
                nc.vector.tensor_copy(u_rows[:, H:H + 1], ptc[:])

                # per segment: stage u-row to partition 0, PE-broadcast to
                # PSUM, ACT-copy to SBUF, then one fused dot per block.
                for sl in range(GSEG):
                    s = s0 + sl
                    ustage = ustagep.tile([1, W1], F32, tag="ustage")
                    nc.scalar.dma_start(ustage[:], u_rows[sl:sl + 1, :])
                    pub = ubcp.tile([P, W1], F32, tag="ubc")
                    nc.tensor.matmul(
                        out=pub[:], lhsT=ones1[:], rhs=ustage[:1, :],
                        start=True, stop=True,
                    )
                    ubs = ubcsbp.tile([P, W1], F32, tag="ubs")
                    nc.scalar.copy(out=ubs[:], in_=pub[:])
                    blocks = [("pos", sl, s)]
                    for q in range(NEG_RATIO * s, NEG_RATIO * (s + 1)):
                        blocks.append(("neg", q, POS_BLK + q))
                    for kind, b, col in blocks:
                        if kind == "pos":
                            in0 = pt[:, b * H:(b + 1) * H]
                        else:
                            in0 = neg_tiles[b // GB][
                                :, (b % GB) * H:(b % GB + 1) * H]
                        sc = scratch.tile([P, H], F32, tag="ttr")
                        nc.vector.tensor_tensor_reduce(
                            out=sc[:],
                            in0=in0,
                            in1=ubs[:, 0:H],
                            scale=1.0,
                            scalar=ubs[:, H:H + 1],
                            op0=mybir.AluOpType.mult,
                            op1=mybir.AluOpType.add,
                            accum_out=logits_sb[:, col:col + 1],
                        )

            nc.sync.dma_start(logits_d[:, :], logits_sb[:])

    nc.compile()
    return nc


def get_module() -> bass.Bass:
    global _CACHED
    if _CACHED is None:
        _CACHED = _build_module()
    return _CACHED


def make_in_maps(inputs: dict) -> list[dict]:
    emb = np.ascontiguousarray(np.asarray(inputs["embedding"], dtype=np.float32))
    gs = np.asarray(inputs["grid_sizes"]).astype(np.int64)
    pos_s = np.asarray(inputs["pos_samples"]).astype(np.int32)
    neg_s = np.asarray(inputs["neg_samples"]).astype(np.int32)
    W_i = np.asarray(inputs["W_i"], dtype=np.float32)
    b_i = np.asarray(inputs["b_i"], dtype=np.float32)
    Wb = np.asarray(inputs["W_k"], dtype=np.float32)[0]
    b_kv = np.asarray(inputs["b_k"], dtype=np.float32)

    if not (gs.shape == (N_SEG,) and np.all(gs == SEG_SZ)):
        raise RuntimeError("kernel assumes grid_sizes == 128 everywhere")
    assert pos_s.shape == (N_POS,) and neg_s.shape == (N_NEG,)

    w_iT_np = np.ascontiguousarray(
        W_i.T.reshape(2, P, H).transpose(1, 0, 2))
    wbT_np = np.ascontiguousarray(
        Wb.T.reshape(2, P, H).transpose(1, 0, 2))
    W_ext = np.concatenate([W_i, b_i[:, None]], axis=1)        # [256, 257]
    w_ext_np = np.ascontiguousarray(
        W_ext.reshape(2, P, H + 1).transpose(1, 0, 2))
    b_i2_np = np.ascontiguousarray(b_i.reshape(2, P).T)
    b_k_np = b_kv.reshape(1, 1)

    in_maps = []
    for k in range(N_CORES):
        # natural processing order: pos rows then neg rows of this core
        full = np.concatenate([
            pos_s[k * POS_PC:(k + 1) * POS_PC],
            neg_s[k * NEG_PC:(k + 1) * NEG_PC],
        ])
        sub_table = np.zeros((N_WIN * WIN_POS, H), np.float32)
        idx16_np = np.zeros((P, N_CALLS * IDX_COLS), np.int16)
        for w in range(N_WIN):
            seg = full[w * WIN_POS:(w + 1) * WIN_POS]
            uniq, inv = np.unique(seg, return_inverse=True)
            sub_table[w * WIN_POS:w * WIN_POS + len(uniq)] = emb[uniq]
            # wrapped int16 layout: index i -> partition i%16, column i//16,
            # replicated across the 8 Q7 cores (partition groups of 16).
            wrapped = inv.astype(np.int16).reshape(
                CALLS_PER_WIN, IDX_COLS, 16).transpose(2, 0, 1).reshape(
                16, CALLS_PER_WIN * IDX_COLS)
            cols = slice(w * CALLS_PER_WIN * IDX_COLS,
                         (w + 1) * CALLS_PER_WIN * IDX_COLS)
            idx16_np[:, cols] = np.tile(wrapped, (8, 1))
        inv_k = np.broadcast_to(
            (1.0 / gs[k * SEG_PC:(k + 1) * SEG_PC].astype(np.float64)
             ).astype(np.float32)[None, :], (P, SEG_PC))
        in_maps.append({
            "table": sub_table,
            "idx16": idx16_np,
            "w_iT": w_iT_np,
            "wbT": wbT_np,
            "w_ext": w_ext_np,
            "b_i2": b_i2_np,
            "b_k": b_k_np,
            "inv_sz": np.ascontiguousarray(inv_k),
        })
    return in_maps


def assemble_output(core_outs: list[np.ndarray]) -> np.ndarray:
    pos_parts, neg_parts = [], []
    for k in range(N_CORES):
        o = np.asarray(core_outs[k])
        assert o.shape == (P, TOT_BLK)
        pos_parts.append(o[:, :POS_BLK].T.ravel())
        neg_parts.append(o[:, POS_BLK:].T.ravel())
    return np.concatenate(pos_parts + neg_parts).astype(np.float32)


def kernel(**inputs) -> np.ndarray:
    nc = get_module()
    in_maps = make_in_maps(inputs)
    res = bass_utils.run_bass_kernel_spmd(
        nc, in_maps, core_ids=list(range(N_CORES)))
    return assemble_output([r["logits"] for r in res.results])
